# revision 1
# baseline (speedup 1.0000x reference)
"""BinaryLinear kernel for 8x TRN2 NeuronCores.

out = x @ (weight > 0)  with x [8192, 2048] f32, weight [2048, 2048] f32.

Sharding: data-parallel over batch (1024 rows/core), weight replicated.

Per core (M=1024, K=2048, N=2048), measured ~170-180us on HW, PE-bound:
- x loads naturally [128, 2048] f32 (two half-row DMAs for early start),
  is cast to bf16 on DVE in 512-col chunks (round-to-nearest), and
  transposed 128x128-blockwise on the PE (is_transpose matmul vs
  identity) so n_in lands on partitions; PSUM results are copied back to
  SBUF by the ACT engine. The DMA XBAR transpose path is deliberately
  avoided: DMATranspose mixed with regular DMACopy serializes on every
  xbar-mode transition within a queue (HW bug workaround), ~1.4-3.7us
  each, and a transpose-only-queue variant convoys even worse end to end.
- weight streams kt-major (full f32 rows) and is binarized to exact bf16
  {0,1} on DVE in 512-col chunks. x is emitted before w so the transpose
  pipeline starts immediately; weights overlap the transpose phase.
- matmul: out[bt, nt] += xT[kt,bt].T @ wbin[kt][:, nt] accumulates all 16
  kt in PSUM (bf16 PE rate, fp32 accumulation). PSUM groups are nt-pairs
  (2 banks) so banks recycle fast; evictions split across ACT and DVE.
  Transposes for bt+2 are emitted between mm-blocks so they run at the
  HAM-warm clock and fill PE gaps. Redundant per-matmul LDWEIGHTS of an
  already-loaded stationary tile are stripped post-schedule (walrus
  ldw-opt is disabled, so tile_legalize emits one per matmul).
"""

import numpy as np

import concourse.bass as bass
import concourse.mybir as mybir
import concourse.tile as tile
from concourse import bacc
from concourse.bass_utils import run_bass_kernel_spmd
from concourse.masks import make_identity

B, K, N = 8192, 2048, 2048
N_CORES = 8
MB = B // N_CORES          # 1024 batch rows per core
P = 128
KT = K // P                # 16 k-tiles
BT = MB // P               # 8 batch tiles per core
NT = 4                     # output column blocks
NB = N // NT               # 512
KG = 4                     # k-tiles per weight DMA

F32 = mybir.dt.float32
BF16 = mybir.dt.bfloat16


def build_kernel(repeat: int = 1, mode: str = "full"):
    nc = bacc.Bacc(None, target_bir_lowering=False)
    x = nc.dram_tensor("x", [MB, K], F32, kind="ExternalInput")
    w = nc.dram_tensor("w", [K, N], F32, kind="ExternalInput")
    out = nc.dram_tensor("out", [MB, N], F32, kind="ExternalOutput")

    w3 = w[:].rearrange("(kt p) n -> p kt n", p=P)   # [128, 16, 2048]

    def body(tc, pools):
        (xraw_pool, xbf_pool, xT_pool, wraw_pool, wbin_pool,
         out_pool, psum_pool, tpsum_pool, const_pool) = pools
        do_x = mode in ("full", "nomm", "xonly")
        do_w = mode in ("full", "nomm", "wonly")
        do_mm = mode in ("full", "mmonly")

        ident = const_pool.tile([P, P], BF16, tag="ident", name="ident")
        make_identity(nc, ident)

        # ---- x loads + casts ----
        xbf = {}
        xT = {}
        if do_x:
            for bt in range(BT):
                xr = xraw_pool.tile([P, K], F32, tag="xraw", name="xr")
                xb = xbf_pool.tile([P, K], BF16, tag=f"xbf_{bt}",
                                   name=f"xbf_{bt}")
                for h in range(2):
                    nc.sync.dma_start(
                        xr[:, h * (K // 2):(h + 1) * (K // 2)],
                        x[bt * P:(bt + 1) * P, h * (K // 2):(h + 1) * (K // 2)])
                for c in range(4):
                    nc.vector.tensor_copy(
                        xb[:, c * NB:(c + 1) * NB], xr[:, c * NB:(c + 1) * NB])
                xbf[bt] = xb

        # ---- weight first (DMA priority): kt-major rows, binarize ----
        wbin = {}
        for kt in range(KT):
            wb = wbin_pool.tile([P, N], BF16, tag=f"wbin_{kt}",
                                name=f"wbin_{kt}")
            if do_w:
                wr = wraw_pool.tile([P, N], F32, tag="wraw", name="wr")
                nc.sync.dma_start(wr[:], w3[:, kt, :])
                for c in range(NT):
                    nc.vector.tensor_scalar(
                        out=wb[:, c * NB:(c + 1) * NB],
                        in0=wr[:, c * NB:(c + 1) * NB],
                        scalar1=0.0, scalar2=None,
                        op0=mybir.AluOpType.is_gt)
            else:
                nc.any.memset(wb[:], 1.0)
            wbin[kt] = wb

        def emit_transposes(bt):
            for kt in range(KT):
                if do_x:
                    tp = tpsum_pool.tile([P, P], BF16, tag="tpsum", name="tp")
                    nc.tensor.transpose(tp[:], xbf[bt][:, kt * P:(kt + 1) * P],
                                        ident[:])
                    t = xT_pool.tile([P, P], BF16, tag=f"xT_{kt}_{bt}",
                                     name=f"xT_{kt}_{bt}")
                    nc.scalar.activation(
                        t[:], tp[:], mybir.ActivationFunctionType.Copy)
                else:
                    t = xT_pool.tile([P, P], BF16, tag=f"xT_{kt}_{bt}",
                                     name=f"xT_{kt}_{bt}")
                    nc.any.memset(t[:], 1.0)
                xT[kt, bt] = t

        def emit_mm_block(bt):
            for pair in range(2):
                nts = (2 * pair, 2 * pair + 1)
                pss = {nt: psum_pool.tile([P, NB], F32, tag="ps", name="ps")
                       for nt in nts}
                for kt in range(KT):
                    for nt in nts:
                        nc.tensor.matmul(
                            pss[nt][:], xT[kt, bt][:],
                            wbin[kt][:, nt * NB:(nt + 1) * NB],
                            start=(kt == 0), stop=(kt == KT - 1))
                for j, nt in enumerate(nts):
                    ot = out_pool.tile([P, NB], F32, tag="osb", name="ot")
                    if j == 0:
                        nc.scalar.activation(
                            ot[:], pss[nt][:],
                            mybir.ActivationFunctionType.Copy)
                    else:
                        nc.vector.tensor_copy(ot[:], pss[nt][:])
                    nc.sync.dma_start(
                        out[bt * P:(bt + 1) * P, nt * NB:(nt + 1) * NB],
                        ot[:])

        # transposes for the first two batch tiles up front, then pipeline:
        # mm(bt) runs while T(bt+2) fills PE stalls at warm clock.
        emit_transposes(0)
        emit_transposes(1)
        if do_mm:
            for bt in range(BT):
                emit_mm_block(bt)
                if bt + 2 < BT:
                    emit_transposes(bt + 2)
        else:
            for bt in range(2, BT):
                emit_transposes(bt)

    with tile.TileContext(nc) as tc:
        with (
            tc.tile_pool(name="xraw", bufs=2) as xraw_pool,
            tc.tile_pool(name="xbf", bufs=1) as xbf_pool,
            tc.tile_pool(name="xT", bufs=1) as xT_pool,
            tc.tile_pool(name="wraw", bufs=3) as wraw_pool,
            tc.tile_pool(name="wbin", bufs=1) as wbin_pool,
            tc.tile_pool(name="osb", bufs=6) as out_pool,
            tc.tile_pool(name="ps", bufs=6, space="PSUM") as psum_pool,
            tc.tile_pool(name="tps", bufs=2, space="PSUM") as tpsum_pool,
            tc.tile_pool(name="const", bufs=1) as const_pool,
        ):
            pools = (xraw_pool, xbf_pool, xT_pool, wraw_pool, wbin_pool,
                     out_pool, psum_pool, tpsum_pool, const_pool)
            if repeat == 1:
                body(tc, pools)
            else:
                with tc.For_i(0, repeat, 1):
                    body(tc, pools)
    _dedup_ldweights(nc)
    nc.compile()
    return nc


def _ldw_key(ins):
    ap = ins.ins[0]
    bap = getattr(ap, "bass_ap", None)
    return (getattr(ap, "memref", None), getattr(bap, "offset", None),
            str(getattr(bap, "ap", None)), getattr(ins, "is_transpose", None))


def _dedup_ldweights(nc):
    """Remove PE weight reloads of the already-loaded stationary operand.

    tile_legalize emits one InstLdweights per InstMatmult even when
    consecutive matmuls share the stationary tile (walrus ldw-opt is
    disabled). The PE executes in stream order, so an InstLdweights whose
    weights AP equals the previous load (with no intervening load) is a
    no-op on the array state and can be dropped. Only waitless/updateless
    loads are dropped, so no synchronization is lost."""
    removed = 0
    for bb in nc.main_func.blocks:
        il = bb.instructions
        last_key = None
        drop = []
        for idx, ins in enumerate(il):
            if not isinstance(ins, mybir.InstLdweights):
                continue
            si = ins.sync_info
            has_sync = si is not None and (
                (si.on_wait and len(si.on_wait) > 0)
                or (si.on_update and len(si.on_update) > 0))
            key = _ldw_key(ins)
            if key == last_key and not has_sync:
                drop.append(idx)
                removed += 1
            else:
                last_key = key
        for idx in reversed(drop):
            del il[idx]
    return removed


_NC_CACHE = None


def _get_nc():
    global _NC_CACHE
    if _NC_CACHE is None:
        _NC_CACHE = build_kernel()
    return _NC_CACHE


def kernel(x: np.ndarray, weight: np.ndarray):
    assert x.shape == (B, K) and weight.shape == (K, N)
    x = np.ascontiguousarray(x, dtype=np.float32)
    weight = np.ascontiguousarray(weight, dtype=np.float32)
    nc = _get_nc()
    in_maps = [
        {"x": x[i * MB:(i + 1) * MB], "w": weight}
        for i in range(N_CORES)
    ]
    res = run_bass_kernel_spmd(nc, in_maps, core_ids=list(range(N_CORES)))
    return np.concatenate([res.results[i]["out"] for i in range(N_CORES)], axis=0)

